# revision 13
# baseline (speedup 1.0000x reference)
"""Trainium2 Bass kernel for nn_AttentiveAutoEncoder.

Key structure: the input embedding is Linear(1,E), so the token embedding
h[b,f,:] = x[b,f] * W_emb[0,:] is rank-1.  All of q/k/v and the MHA in_proj
outputs are therefore affine in the scalar x[b,f]:

    q2[b,f,:] = x[b,f]*u_q + c_q       (u_q, c_q host-precomputed [H])

so per head the attention scores collapse to

    scores[b,h,i,j] = (a_h x_i + c_h) x_j  +  (terms constant in j)

and the j-constant terms drop out of the softmax.  The context vector
collapses to ctx[b,i,head] = s[b,h,i] * u_v[head] + c_v with
s[b,h,i] = sum_j attn[b,h,i,j] x[b,j], so the whole attention block plus
out-proj becomes a [*,NH] @ [NH,H] matmul.  Only the per-feature grouped
MLP stacks remain as real GEMM work; they run on the TensorEngine with
transposed activations (zT layout: [H, tokens]) so the layer output layout
equals the next layer's input layout and no per-layer transposes are needed.

Sharding: pure data-parallel over batch B across the 8 cores (no
collectives); the grouped-GEMM weights are replicated to every core.
"""

import numpy as np

B, F, E, H, NH, OUT, NL = 2048, 64, 8, 256, 4, 64, 4
HD = H // NH
NCORES = 8
BL = B // NCORES          # 256 local batch rows per core
BT = 128                  # batch tile (partition dim)
NBT = BL // BT            # 2 batch tiles per core
P = 128
FQ = F // 4               # features per chunk (aT/sT SBUF budget)


def _host_precompute(inp):
    """Collapse the attention block into a handful of small tensors."""
    f64 = lambda k: np.asarray(inp[k], dtype=np.float64)
    W_emb, Wq, bq = f64("W_emb"), f64("Wq"), f64("bq")
    Wk, bk, Wv, bv = f64("Wk"), f64("bk"), f64("Wv"), f64("bv")
    Win, bin_, Wo, bo = f64("Win"), f64("bin_"), f64("Wo"), f64("bo")
    Wq2, Wk2, Wv2 = np.split(Win, 3, axis=0)
    bq2, bk2, bv2 = np.split(bin_, 3)
    e = W_emb[0]
    uq = (e @ Wq) @ Wq2.T
    cq = bq @ Wq2.T + bq2
    uk = (e @ Wk) @ Wk2.T
    uv = (e @ Wv) @ Wv2.T
    cv = bv @ Wv2.T + bv2
    ck = f64("bk") @ Wk2.T + bk2
    sc = 1.0 / np.sqrt(HD)
    ah = np.array([uq[h * HD:(h + 1) * HD] @ uk[h * HD:(h + 1) * HD]
                   for h in range(NH)]) * sc
    ch = np.array([cq[h * HD:(h + 1) * HD] @ uk[h * HD:(h + 1) * HD]
                   for h in range(NH)]) * sc
    # a[b,i,:] = sum_h s[b,h,i] * Mproj[h,:] + const_a
    Mproj = np.stack([uv[h * HD:(h + 1) * HD] @ Wo[:, h * HD:(h + 1) * HD].T
                      for h in range(NH)])          # [NH, H]
    const_a = cv @ Wo.T + bo                        # [H]
    return (ah.astype(np.float32), ch.astype(np.float32),
            Mproj.astype(np.float32), const_a.astype(np.float32))


def _build_graph():
    import concourse.bass as bass
    import concourse.mybir as mybir
    import concourse.tile as tile
    from concourse import bacc
    from concourse.masks import make_identity

    f32 = mybir.dt.float32
    bf16 = mybir.dt.bfloat16
    AF = mybir.ActivationFunctionType
    ALU = mybir.AluOpType
    AXL = mybir.AxisListType

    nc = bacc.Bacc(None)

    x_d = nc.declare_dram_parameter("xs", [BL, F], f32, isOutput=False)
    attc_d = nc.declare_dram_parameter("attc", [2 * NH], f32, isOutput=False)
    mproj_d = nc.declare_dram_parameter("mproj", [NH, H], bf16, isOutput=False)
    ca_d = nc.declare_dram_parameter("consta", [H], f32, isOutput=False)
    wenc_d = nc.declare_dram_parameter("wenc", [F, NL, H, H], bf16, isOutput=False)
    benc_d = nc.declare_dram_parameter("benc", [F, NL, H], f32, isOutput=False)
    wdec_d = nc.declare_dram_parameter("wdec", [F, NL, H, H], bf16, isOutput=False)
    bdec_d = nc.declare_dram_parameter("bdec", [F, NL, H], f32, isOutput=False)
    wout_d = nc.declare_dram_parameter("wout", [H, OUT], bf16, isOutput=False)
    bout_d = nc.declare_dram_parameter("bout", [OUT], f32, isOutput=False)
    pc_d = nc.declare_dram_parameter("out_pc", [BL, F, OUT], f32, isOutput=True)
    attn_d = nc.declare_dram_parameter("out_attn", [BL, NH, F, F], f32,
                                       isOutput=True)
    # DRAM bounce for the s transpose ([b-major] -> [nh, i, b])
    s2_d = nc.dram_tensor("s_bounce", [NH, F, BL], bf16)

    with tile.TileContext(nc) as tc:
        with (
            tc.tile_pool(name="singles", bufs=1) as singles,
            tc.tile_pool(name="att", bufs=2) as att,
            tc.tile_pool(name="att1", bufs=1) as att1,
            tc.tile_pool(name="attsm", bufs=2) as attsm,
            tc.tile_pool(name="stp", bufs=2) as stpool,
            tc.tile_pool(name="aT", bufs=1) as aT_pool,
            tc.tile_pool(name="wpool", bufs=6) as wpool,
            tc.tile_pool(name="zpool", bufs=2) as zpool,
            tc.tile_pool(name="opool", bufs=3) as opool,
            tc.tile_pool(name="ps_mm", bufs=2, space="PSUM") as ps_mm,
            tc.tile_pool(name="ps_g", bufs=2, space="PSUM") as ps_g,
            tc.tile_pool(name="ps_o", bufs=2, space="PSUM") as ps_o,
            tc.tile_pool(name="ps_t", bufs=2, space="PSUM") as ps_t,
        ):
            # ---- one-time setup ------------------------------------------
            attc_t = singles.tile([P, 2 * NH], f32)
            nc.sync.dma_start(out=attc_t,
                              in_=attc_d[:].partition_broadcast(P))
            x_t = singles.tile([P, NBT, F], f32)
            nc.sync.dma_start(
                out=x_t, in_=x_d[:, :].rearrange("(t p) f -> p t f", p=P))
            mproj_t = singles.tile([NH, H], bf16)
            nc.sync.dma_start(out=mproj_t, in_=mproj_d[:, :])
            ca_t = singles.tile([P, 2], f32)
            nc.sync.dma_start(out=ca_t,
                              in_=ca_d[:].rearrange("(c p) -> p c", p=P))
            wout_t = singles.tile([P, 2, OUT], bf16)
            nc.sync.dma_start(
                out=wout_t, in_=wout_d[:, :].rearrange("(c p) o -> p c o", p=P))
            bout_t = singles.tile([OUT, 1], f32)
            nc.sync.dma_start(out=bout_t,
                              in_=bout_d[:].rearrange("(o u) -> o u", u=1))
            ident = singles.tile([OUT, OUT], f32)
            make_identity(nc, ident)

            # s for all heads, both b tiles: [128, NBT, NH, F]
            s_t = singles.tile([P, NBT, NH, F], bf16)

            # prime DVE's vector clock on the setup DMAs so the 1-wait-slot
            # TensorTensor instructions below never need >1 sync wait
            prime_t = singles.tile([P, 2], f32)
            nc.vector.tensor_copy(prime_t[:, 0:1], x_t[:, 0, 0:1])
            nc.vector.tensor_copy(prime_t[:, 1:2], attc_t[:, 0:1])

            # ---- attention (collapsed) -----------------------------------
            for bt in range(NBT):
                xv = x_t[:, bt, :]                       # [128, F]
                # alpha[p, h, i] = ah[h] * x[p, i] + ch[h]
                alpha_t = att.tile([P, NH, F], f32, tag="alpha")
                nc.vector.tensor_tensor(
                    out=alpha_t,
                    in0=xv.unsqueeze(1).broadcast_to([P, NH, F]),
                    in1=attc_t[:, 0:NH].unsqueeze(2).broadcast_to([P, NH, F]),
                    op=ALU.mult,
                )
                nc.vector.tensor_tensor(
                    out=alpha_t,
                    in0=alpha_t,
                    in1=attc_t[:, NH:2 * NH].unsqueeze(2).broadcast_to([P, NH, F]),
                    op=ALU.add,
                )
                for q in range(NH):
                    # S[p, i, j] = alpha[p, q, i] * x[p, j]
                    S_t = att1.tile([P, F, F], f32, tag="S")
                    nc.vector.tensor_tensor(
                        out=S_t,
                        in0=alpha_t[:, q, :].unsqueeze(2).broadcast_to([P, F, F]),
                        in1=xv.unsqueeze(1).broadcast_to([P, F, F]),
                        op=ALU.mult,
                    )
                    # E = exp(S)
                    E_t = att.tile([P, F, F], f32, tag="E")
                    nc.scalar.activation(E_t, S_t, AF.Exp)
                    # D = sum_j E ; numer = sum_j E*x_j
                    D_t = attsm.tile([P, F], f32, tag="D")
                    nc.vector.tensor_reduce(
                        out=D_t, in_=E_t, axis=AXL.X, op=ALU.add)
                    T_t = att1.tile([P, F, F], f32, tag="T")
                    nc.vector.tensor_tensor(
                        out=T_t, in0=E_t,
                        in1=xv.unsqueeze(1).broadcast_to([P, F, F]),
                        op=ALU.mult,
                    )
                    N_t = attsm.tile([P, F], f32, tag="N")
                    nc.vector.tensor_reduce(
                        out=N_t, in_=T_t, axis=AXL.X, op=ALU.add)
                    rD_t = attsm.tile([P, F], f32, tag="rD")
                    nc.vector.reciprocal(rD_t, D_t)
                    # attn = E * rD  (scalar_tensor_tensor: its instruction
                    # struct has more sync-wait slots than TensorTensor)
                    A_t = att.tile([P, F, F], f32, tag="A")
                    nc.vector.scalar_tensor_tensor(
                        out=A_t, in0=E_t, scalar=1.0,
                        in1=rD_t.unsqueeze(2).broadcast_to([P, F, F]),
                        op0=ALU.mult, op1=ALU.mult,
                    )
                    nc.sync.dma_start(
                        out=attn_d[bt * BT:(bt + 1) * BT, q, :, :], in_=A_t)
                    # s = numer * rD
                    nc.vector.tensor_tensor(
                        out=s_t[:, bt, q, :], in0=N_t, in1=rD_t, op=ALU.mult)
                    # bounce s block to DRAM transposed: s2[q, i, b] (b fast)
                    nc.sync.dma_start(
                        out=s2_d[q, :, :].transpose([1, 0])[bt * BT:(bt + 1) * BT, :],
                        in_=s_t[:, bt, q, :],
                    )
            # ---- per f-chunk: M-projection then grouped MLP stacks -------
            NCH = 512
            for fc in range(F // FQ):
                # sT chunk in matmul-rhs layout: [NH, FQ, BL]
                sT_t = stpool.tile([NH, FQ, BL], bf16, tag="sT")
                nc.sync.dma_start(
                    out=sT_t, in_=s2_d[:, fc * FQ:(fc + 1) * FQ, :])
                # aT[o(pc), hc, f_local, b] for this chunk
                aT_t = aT_pool.tile([P, 2, FQ, BL], bf16, tag="aT")
                aT_flat = aT_t.rearrange("p c f b -> p c (f b)")
                sT_flat = sT_t.rearrange("h i b -> h (i b)")
                for hc in range(2):
                    for n0 in range(0, FQ * BL, NCH):
                        mm_ps = ps_mm.tile([P, NCH], f32, tag="mm")
                        nc.tensor.matmul(
                            mm_ps,
                            mproj_t[:, hc * P:(hc + 1) * P],
                            sT_flat[:, n0:n0 + NCH],
                            start=True, stop=True,
                        )
                        nc.scalar.activation(
                            aT_flat[:, hc, n0:n0 + NCH],
                            mm_ps, AF.Identity, bias=ca_t[:, hc:hc + 1])

                for fl in range(FQ):
                    f = fc * FQ + fl
                    zT = aT_t[:, :, fl, :]                   # [128, 2, BL]
                    for l in range(2 * NL):
                        w_d = wenc_d if l < NL else wdec_d
                        b_d = benc_d if l < NL else bdec_d
                        lj = l if l < NL else l - NL
                        w_t = wpool.tile([P, 2, 2, P], bf16, tag="w")
                        nc.sync.dma_start(
                            out=w_t,
                            in_=w_d[f, lj].rearrange(
                                "(c p) (m q) -> p c m q", p=P, q=P),
                        )
                        b_t = wpool.tile([P, 2], f32, tag="b")
                        nc.sync.dma_start(
                            out=b_t,
                            in_=b_d[f, lj].rearrange("(c p) -> p c", p=P))
                        zT_next = zpool.tile([P, 2, BL], bf16, tag="z")
                        for m in range(2):
                            g_ps = ps_g.tile([P, BL], f32, tag="g")
                            nc.tensor.matmul(
                                g_ps,
                                w_t[:, 0, m, :],
                                zT[:, 0, :],
                                start=True, stop=False)
                            nc.tensor.matmul(
                                g_ps,
                                w_t[:, 1, m, :],
                                zT[:, 1, :],
                                start=False, stop=True)
                            nc.scalar.activation(
                                zT_next[:, m, :], g_ps, AF.Relu,
                                bias=b_t[:, m:m + 1])
                        zT = zT_next

                    # out-proj + sigmoid: [64, BL]
                    o_ps = ps_o.tile([OUT, BL], f32, tag="o")
                    nc.tensor.matmul(
                        o_ps, wout_t[:, 0, :],
                        zT[:, 0, :], start=True, stop=False)
                    nc.tensor.matmul(
                        o_ps, wout_t[:, 1, :],
                        zT[:, 1, :], start=False, stop=True)
                    sig_t = opool.tile([OUT, BL], f32, tag="sig")
                    nc.scalar.activation(sig_t, o_ps, AF.Sigmoid,
                                         bias=bout_t[:, 0:1])
                    # transpose [64, 128] -> [128, 64] per b tile, DMA out
                    for bt in range(NBT):
                        t_ps = ps_t.tile([P, OUT], f32, tag="t")
                        nc.tensor.transpose(
                            t_ps, sig_t[:, bt * BT:(bt + 1) * BT], ident)
                        oc_t = opool.tile([P, OUT], f32, tag="oc")
                        nc.vector.tensor_copy(oc_t, t_ps)
                        nc.sync.dma_start(
                            out=pc_d[bt * BT:(bt + 1) * BT, f, :], in_=oc_t)

    nc.compile()
    return nc


def kernel(**inputs):
    import sys
    for p in ("/opt/trn_rl_repo", "/opt/pypackages"):
        if p not in sys.path:
            sys.path.insert(0, p)
    from concourse.bass_utils import run_bass_kernel_spmd
    import ml_dtypes

    bf = ml_dtypes.bfloat16
    ah, ch, Mproj, const_a = _host_precompute(inputs)
    attc = np.concatenate([ah, ch]).astype(np.float32)

    x = np.ascontiguousarray(np.asarray(inputs["x"], dtype=np.float32))
    shared = {
        "attc": attc,
        "mproj": np.ascontiguousarray(Mproj.astype(bf)),
        "consta": np.ascontiguousarray(const_a),
        "wenc": np.ascontiguousarray(np.asarray(inputs["Wenc"], np.float32).astype(bf)),
        "benc": np.ascontiguousarray(np.asarray(inputs["benc"], np.float32)),
        "wdec": np.ascontiguousarray(np.asarray(inputs["Wdec"], np.float32).astype(bf)),
        "bdec": np.ascontiguousarray(np.asarray(inputs["bdec"], np.float32)),
        "wout": np.ascontiguousarray(np.asarray(inputs["Wout"], np.float32).astype(bf)),
        "bout": np.ascontiguousarray(np.asarray(inputs["bout"], np.float32)),
    }
    in_maps = [
        {"xs": np.ascontiguousarray(x[i * BL:(i + 1) * BL]), **shared}
        for i in range(NCORES)
    ]

    nc = _build_graph()
    res = run_bass_kernel_spmd(nc, in_maps, core_ids=list(range(NCORES)))
    pc = np.concatenate([res.results[i]["out_pc"] for i in range(NCORES)], axis=0)
    attn = np.concatenate([res.results[i]["out_attn"] for i in range(NCORES)],
                          axis=0)
    return pc.astype(np.float32), attn.astype(np.float32)
